# revision 9
# baseline (speedup 1.0000x reference)
"""Trainium2 Bass kernel for CeptaSSMLiteLowRank.

Computes, per batch b (B=8, T=2048, P=1024, P_r=64):
    r   = t @ V_r                                  (B,T,64)
    a   = clip(sigmoid(r @ W_w.T + W_b), .01, .995)
    inj = (F*t) @ V_b
    s_t = a_t * s_{t-1} + inj_t   (scan over T)
    out = s @ V_o, plus s_last

Sharding: data-parallel over B across 8 NeuronCores (1 batch/core), params
replicated. The T-scan runs on the DVE hardware scan op (tensor_tensor_scan)
in [P_r-partition, T-free] layout; the P-contraction matmuls need t (and F*t)
transposed, done via PE-transpose of 128x128 blocks.
"""

import sys

if "/opt/trn_rl_repo" not in sys.path:
    sys.path.insert(0, "/opt/trn_rl_repo")

import numpy as np

import concourse.bacc as bacc
import concourse.bass as bass
import concourse.tile as tile
from concourse import mybir
from concourse.bass_utils import run_bass_kernel_spmd
from concourse.masks import make_identity

B, T, P, PR = 8, 2048, 1024, 64
A_MIN, A_MAX = 0.01, 0.995

F32 = mybir.dt.float32
TC = 128           # timestep tile (partition dim of natural tiles)
TB = 512           # timestep block (scan/matmul granularity)
NPC = P // 128     # 8 P-chunks
NB = T // TB       # 4 blocks
PAIR = 2           # T-tiles per DMA (1 MiB transfers)

LAST_RUN = {}


def _build() -> bass.Bass:
    nc = bacc.Bacc()

    t_d = nc.declare_dram_parameter("t", [T, P], F32, isOutput=False)
    f_d = nc.declare_dram_parameter("F", [T, P], F32, isOutput=False)
    vr_d = nc.declare_dram_parameter("V_r", [P, PR], F32, isOutput=False)
    vb_d = nc.declare_dram_parameter("V_b", [P, PR], F32, isOutput=False)
    vo_d = nc.declare_dram_parameter("V_o", [PR, P], F32, isOutput=False)
    wwt_d = nc.declare_dram_parameter("W_wT", [PR, PR], F32, isOutput=False)
    wb_d = nc.declare_dram_parameter("W_b", [PR, 1], F32, isOutput=False)
    out_d = nc.declare_dram_parameter("t_tilde", [T, P], F32, isOutput=True)
    slast_d = nc.declare_dram_parameter("s_last", [PR, 1], F32, isOutput=True)

    with tile.TileContext(nc) as tc:
        with (
            tc.tile_pool(name="consts", bufs=1) as consts,
            tc.tile_pool(name="inp", bufs=2) as inp,
            tc.tile_pool(name="trans", bufs=2) as trans,
            tc.tile_pool(name="small", bufs=2) as small,
            tc.tile_pool(name="outp", bufs=2) as outp,
            tc.tile_pool(name="ps_tr", bufs=4, space="PSUM") as ps_tr,
            tc.tile_pool(name="ps_mm", bufs=2, space="PSUM") as ps_mm,
            tc.tile_pool(name="ps_out", bufs=1, space="PSUM") as ps_out,
        ):
            ident = consts.tile([128, 128], F32)
            make_identity(nc, ident)

            # V_r/V_b as lhsT chunks: [K=128 (P-chunk), 8 chunks, M=64]
            vr_sb = consts.tile([128, NPC, PR], F32)
            nc.sync.dma_start(
                out=vr_sb, in_=vr_d[:].rearrange("(c k) m -> k c m", k=128)
            )
            vb_sb = consts.tile([128, NPC, PR], F32)
            nc.sync.dma_start(
                out=vb_sb, in_=vb_d[:].rearrange("(c k) m -> k c m", k=128)
            )
            vo_sb = consts.tile([PR, P], F32)
            nc.sync.dma_start(out=vo_sb, in_=vo_d[:])
            wwt_sb = consts.tile([PR, PR], F32)
            nc.sync.dma_start(out=wwt_sb, in_=wwt_d[:])
            wb_sb = consts.tile([PR, 1], F32)
            nc.sync.dma_start(out=wb_sb, in_=wb_d[:])

            # s^T accumulator for the whole sequence: [64, T]
            sT = small.tile([PR, T], F32, bufs=1)

            cp_i = 0  # round-robin PSUM->SBUF copies between DVE and ACT

            def copy_rr(dst, src):
                nonlocal cp_i
                if cp_i % 2 == 0:
                    nc.vector.tensor_copy(dst, src)
                else:
                    nc.scalar.copy(dst, src)
                cp_i += 1

            for b in range(NB):
                t0 = b * TB

                # transposed tiles for this block: [128 (P-chunk part), 8, TB]
                tT = trans.tile([128, NPC, TB], F32, tag="tT")
                gT = trans.tile([128, NPC, TB], F32, tag="gT")

                for j in range(TB // (TC * PAIR)):  # 2 pair-tiles per block
                    row0 = t0 + j * TC * PAIR
                    t_nat = inp.tile([128, PAIR, P], F32, tag="t_nat")
                    nc.sync.dma_start(
                        out=t_nat,
                        in_=t_d[row0 : row0 + TC * PAIR, :].rearrange(
                            "(a k) d -> k a d", k=128
                        ),
                    )
                    g_nat = inp.tile([128, PAIR, P], F32, tag="g_nat")
                    nc.sync.dma_start(
                        out=g_nat,
                        in_=f_d[row0 : row0 + TC * PAIR, :].rearrange(
                            "(a k) d -> k a d", k=128
                        ),
                    )
                    # G = F * t (elementwise) on gpsimd, in place over F
                    nc.gpsimd.tensor_mul(g_nat, g_nat, t_nat)

                    for a in range(PAIR):
                        ti = j * PAIR + a  # T-tile index within block (0..3)
                        for src, dstT, nm in ((t_nat, tT, "t"), (g_nat, gT, "g")):
                            for grp in range(2):  # P-chunks 0-3 and 4-7
                                ps = ps_tr.tile(
                                    [128, 512], F32, tag="ps_tr", name=f"ps_{nm}"
                                )
                                for c4 in range(4):
                                    c = grp * 4 + c4
                                    nc.tensor.transpose(
                                        ps[:, c4 * 128 : (c4 + 1) * 128],
                                        src[:, a, c * 128 : (c + 1) * 128],
                                        ident,
                                    )
                                # one engine per destination tensor, so each
                                # consumer matmul waits on a single semaphore
                                eng = nc.vector if nm == "t" else nc.scalar
                                if nm == "t":
                                    eng.tensor_copy(
                                        dstT[
                                            :,
                                            grp * 4 : (grp + 1) * 4,
                                            ti * 128 : (ti + 1) * 128,
                                        ],
                                        ps[:].rearrange("k (c m) -> k c m", c=4),
                                    )
                                else:
                                    eng.copy(
                                        dstT[
                                            :,
                                            grp * 4 : (grp + 1) * 4,
                                            ti * 128 : (ti + 1) * 128,
                                        ],
                                        ps[:].rearrange("k (c m) -> k c m", c=4),
                                    )

                # r^T and inj^T for this block: accumulate over 8 P-chunks
                rT_ps = ps_mm.tile([PR, TB], F32, tag="mm", name="rT_ps")
                for c in range(NPC):
                    nc.tensor.matmul(
                        rT_ps,
                        lhsT=vr_sb[:, c, :],
                        rhs=tT[:, c, :],
                        start=(c == 0),
                        stop=(c == NPC - 1),
                    )
                rT_sb = small.tile([PR, TB], F32, tag="rT")
                nc.vector.tensor_copy(rT_sb, rT_ps)

                injT_ps = ps_mm.tile([PR, TB], F32, tag="mm", name="injT_ps")
                for c in range(NPC):
                    nc.tensor.matmul(
                        injT_ps,
                        lhsT=vb_sb[:, c, :],
                        rhs=gT[:, c, :],
                        start=(c == 0),
                        stop=(c == NPC - 1),
                    )
                injT_sb = small.tile([PR, TB], F32, tag="injT")
                nc.scalar.copy(injT_sb, injT_ps)

                # a^T = clip(sigmoid(W_w @ r^T + W_b))
                zT_ps = ps_mm.tile([PR, TB], F32, tag="mm", name="zT_ps")
                nc.tensor.matmul(zT_ps, lhsT=wwt_sb, rhs=rT_sb, start=True, stop=True)
                aT = small.tile([PR, TB], F32, tag="aT")
                nc.scalar.activation(
                    aT, zT_ps, mybir.ActivationFunctionType.Sigmoid, bias=wb_sb
                )
                nc.vector.tensor_scalar(
                    aT,
                    aT,
                    float(A_MAX),
                    float(A_MIN),
                    op0=mybir.AluOpType.min,
                    op1=mybir.AluOpType.max,
                )

                # scan: s = a * s_prev + inj along T (chained across blocks)
                init = 0.0 if b == 0 else sT[:, t0 - 1 : t0]
                nc.vector.tensor_tensor_scan(
                    sT[:, t0 : t0 + TB],
                    aT,
                    injT_sb,
                    init,
                    op0=mybir.AluOpType.mult,
                    op1=mybir.AluOpType.add,
                )

                # t_tilde block: (s^T)^T @ V_o, one [128, P] tile per T-chunk
                for j in range(TB // (TC * PAIR)):
                    ps_o = ps_out.tile([128, P], F32, tag="out_ps")
                    ps_o2 = ps_out.tile([128, P], F32, tag="out_ps")
                    ob = outp.tile([128, PAIR, P], F32, tag="ob")
                    for a, pso in ((0, ps_o), (1, ps_o2)):
                        tc0 = t0 + (j * PAIR + a) * TC
                        for h in range(2):
                            nc.tensor.matmul(
                                pso[:, h * 512 : (h + 1) * 512],
                                lhsT=sT[:, tc0 : tc0 + TC],
                                rhs=vo_sb[:, h * 512 : (h + 1) * 512],
                                start=True,
                                stop=True,
                            )
                        if (b * 2 + j) % 2 == 0:
                            nc.vector.tensor_copy(ob[:, a, :], pso)
                        else:
                            nc.scalar.copy(ob[:, a, :], pso)
                    row0 = t0 + j * TC * PAIR
                    nc.sync.dma_start(
                        out=out_d[row0 : row0 + TC * PAIR, :].rearrange(
                            "(a k) d -> k a d", k=128
                        ),
                        in_=ob,
                    )

            nc.sync.dma_start(out=slast_d[:], in_=sT[:, T - 1 : T])

    if not nc.is_finalized():
        nc.finalize()
    return nc


_NC_CACHE = None


def _get_nc():
    global _NC_CACHE
    if _NC_CACHE is None:
        _NC_CACHE = _build()
    return _NC_CACHE


def kernel(t, F, V_r, V_b, V_o, W_w, W_b, _trace=False):
    t = np.ascontiguousarray(np.asarray(t, dtype=np.float32))
    F = np.ascontiguousarray(np.asarray(F, dtype=np.float32))
    V_r = np.ascontiguousarray(np.asarray(V_r, dtype=np.float32))
    V_b = np.ascontiguousarray(np.asarray(V_b, dtype=np.float32))
    V_o = np.ascontiguousarray(np.asarray(V_o, dtype=np.float32))
    W_wT = np.ascontiguousarray(np.asarray(W_w, dtype=np.float32).T)
    W_b = np.ascontiguousarray(np.asarray(W_b, dtype=np.float32).reshape(PR, 1))

    nc = _get_nc()
    in_maps = [
        {
            "t": t[b],
            "F": F[b],
            "V_r": V_r,
            "V_b": V_b,
            "V_o": V_o,
            "W_wT": W_wT,
            "W_b": W_b,
        }
        for b in range(B)
    ]
    res = run_bass_kernel_spmd(nc, in_maps, core_ids=list(range(B)), trace=_trace)
    LAST_RUN["exec_time_ns"] = res.exec_time_ns
    LAST_RUN["results"] = res

    t_tilde = np.stack([res.results[b]["t_tilde"] for b in range(B)])
    s_last = np.stack([res.results[b]["s_last"].reshape(PR) for b in range(B)])
    return t_tilde, s_last


# revision 11
# speedup vs baseline: 31185.1033x; 31185.1033x over previous
"""Trainium2 Bass kernel for CeptaSSMLiteLowRank.

Computes, per batch b (B=8, T=2048, P=1024, P_r=64):
    r   = t @ V_r                                  (B,T,64)
    a   = clip(sigmoid(r @ W_w.T + W_b), .01, .995)
    inj = (F*t) @ V_b
    s_t = a_t * s_{t-1} + inj_t   (scan over T)
    out = s @ V_o, plus s_last

Sharding: data-parallel over B across 8 NeuronCores (1 batch/core), params
replicated. The T-scan runs on the DVE hardware scan op (tensor_tensor_scan)
in [P_r-partition, T-free] layout; the P-contraction matmuls need t (and F*t)
transposed, done via PE-transpose of 128x128 blocks.
"""

import sys

if "/opt/trn_rl_repo" not in sys.path:
    sys.path.insert(0, "/opt/trn_rl_repo")

import numpy as np

import concourse.bacc as bacc
import concourse.bass as bass
import concourse.tile as tile
from concourse import mybir
from concourse.bass_utils import run_bass_kernel_spmd
from concourse.masks import make_identity

B, T, P, PR = 8, 2048, 1024, 64
A_MIN, A_MAX = 0.01, 0.995

F32 = mybir.dt.float32
TC = 128           # timestep tile (partition dim of natural tiles)
TB = 512           # timestep block (scan/matmul granularity)
NPC = P // 128     # 8 P-chunks
NB = T // TB       # 4 blocks
PAIR = 2           # T-tiles per DMA (1 MiB transfers)

LAST_RUN = {}


def _build() -> bass.Bass:
    nc = bacc.Bacc()

    t_d = nc.declare_dram_parameter("t", [T, P], F32, isOutput=False)
    f_d = nc.declare_dram_parameter("F", [T, P], F32, isOutput=False)
    vr_d = nc.declare_dram_parameter("V_r", [P, PR], F32, isOutput=False)
    vb_d = nc.declare_dram_parameter("V_b", [P, PR], F32, isOutput=False)
    vo_d = nc.declare_dram_parameter("V_o", [PR, P], F32, isOutput=False)
    wwt_d = nc.declare_dram_parameter("W_wT", [PR, PR], F32, isOutput=False)
    wb_d = nc.declare_dram_parameter("W_b", [PR, 1], F32, isOutput=False)
    out_d = nc.declare_dram_parameter("t_tilde", [T, P], F32, isOutput=True)
    slast_d = nc.declare_dram_parameter("s_last", [PR, 1], F32, isOutput=True)

    with tile.TileContext(nc) as tc:
        with (
            tc.tile_pool(name="consts", bufs=1) as consts,
            tc.tile_pool(name="inp", bufs=2) as inp,
            tc.tile_pool(name="trans", bufs=2) as trans,
            tc.tile_pool(name="small", bufs=2) as small,
            tc.tile_pool(name="outp", bufs=2) as outp,
            tc.tile_pool(name="ps_tr", bufs=4, space="PSUM") as ps_tr,
            tc.tile_pool(name="ps_mm", bufs=2, space="PSUM") as ps_mm,
            tc.tile_pool(name="ps_out", bufs=1, space="PSUM") as ps_out,
        ):
            ident = consts.tile([128, 128], F32)
            make_identity(nc, ident)

            # V_r/V_b as lhsT chunks: [K=128 (P-chunk), 8 chunks, M=64]
            vr_sb = consts.tile([128, NPC, PR], F32)
            nc.sync.dma_start(
                out=vr_sb, in_=vr_d[:].rearrange("(c k) m -> k c m", k=128)
            )
            vb_sb = consts.tile([128, NPC, PR], F32)
            nc.sync.dma_start(
                out=vb_sb, in_=vb_d[:].rearrange("(c k) m -> k c m", k=128)
            )
            vo_sb = consts.tile([PR, P], F32)
            nc.sync.dma_start(out=vo_sb, in_=vo_d[:])
            wwt_sb = consts.tile([PR, PR], F32)
            nc.sync.dma_start(out=wwt_sb, in_=wwt_d[:])
            wb_sb = consts.tile([PR, 1], F32)
            nc.sync.dma_start(out=wb_sb, in_=wb_d[:])

            # s^T accumulator for the whole sequence: [64, T]
            sT = small.tile([PR, T], F32, bufs=1)

            cp_i = 0  # round-robin PSUM->SBUF copies between DVE and ACT

            def copy_rr(dst, src):
                nonlocal cp_i
                if cp_i % 2 == 0:
                    nc.vector.tensor_copy(dst, src)
                else:
                    nc.scalar.copy(dst, src)
                cp_i += 1

            for b in range(NB):
                t0 = b * TB

                # transposed tiles for this block: [128 (P-chunk part), 8, TB]
                tT = trans.tile([128, NPC, TB], F32, tag="tT")
                gT = trans.tile([128, NPC, TB], F32, tag="gT")

                for j in range(TB // (TC * PAIR)):  # 2 pair-tiles per block
                    row0 = t0 + j * TC * PAIR
                    t_nat = inp.tile([128, PAIR, P], F32, tag="t_nat")
                    nc.sync.dma_start(
                        out=t_nat,
                        in_=t_d[row0 : row0 + TC * PAIR, :].rearrange(
                            "(a k) d -> k a d", k=128
                        ),
                    )
                    g_nat = inp.tile([128, PAIR, P], F32, tag="g_nat")
                    nc.sync.dma_start(
                        out=g_nat,
                        in_=f_d[row0 : row0 + TC * PAIR, :].rearrange(
                            "(a k) d -> k a d", k=128
                        ),
                    )
                    # G = F * t (elementwise) on gpsimd, in place over F
                    nc.gpsimd.tensor_mul(g_nat, g_nat, t_nat)

                    for a in range(PAIR):
                        ti = j * PAIR + a  # T-tile index within block (0..3)
                        for src, dstT, nm in ((t_nat, tT, "t"), (g_nat, gT, "g")):
                            for grp in range(2):  # P-chunks 0-3 and 4-7
                                ps = ps_tr.tile(
                                    [128, 512], F32, tag="ps_tr", name=f"ps_{nm}"
                                )
                                for c4 in range(4):
                                    c = grp * 4 + c4
                                    nc.tensor.transpose(
                                        ps[:, c4 * 128 : (c4 + 1) * 128],
                                        src[:, a, c * 128 : (c + 1) * 128],
                                        ident,
                                    )
                                # one engine per destination tensor, so each
                                # consumer matmul waits on a single semaphore
                                eng = nc.vector if nm == "t" else nc.scalar
                                if nm == "t":
                                    eng.tensor_copy(
                                        dstT[
                                            :,
                                            grp * 4 : (grp + 1) * 4,
                                            ti * 128 : (ti + 1) * 128,
                                        ],
                                        ps[:].rearrange("k (c m) -> k c m", c=4),
                                    )
                                else:
                                    eng.copy(
                                        dstT[
                                            :,
                                            grp * 4 : (grp + 1) * 4,
                                            ti * 128 : (ti + 1) * 128,
                                        ],
                                        ps[:].rearrange("k (c m) -> k c m", c=4),
                                    )

                # r^T and inj^T for this block: accumulate over 8 P-chunks
                rT_ps = ps_mm.tile([PR, TB], F32, tag="mm", name="rT_ps")
                for c in range(NPC):
                    nc.tensor.matmul(
                        rT_ps,
                        lhsT=vr_sb[:, c, :],
                        rhs=tT[:, c, :],
                        start=(c == 0),
                        stop=(c == NPC - 1),
                    )
                rT_sb = small.tile([PR, TB], F32, tag="rT")
                nc.vector.tensor_copy(rT_sb, rT_ps)

                injT_ps = ps_mm.tile([PR, TB], F32, tag="mm", name="injT_ps")
                for c in range(NPC):
                    nc.tensor.matmul(
                        injT_ps,
                        lhsT=vb_sb[:, c, :],
                        rhs=gT[:, c, :],
                        start=(c == 0),
                        stop=(c == NPC - 1),
                    )
                injT_sb = small.tile([PR, TB], F32, tag="injT")
                nc.scalar.copy(injT_sb, injT_ps)

                # a^T = clip(sigmoid(W_w @ r^T + W_b))
                zT_ps = ps_mm.tile([PR, TB], F32, tag="mm", name="zT_ps")
                nc.tensor.matmul(zT_ps, lhsT=wwt_sb, rhs=rT_sb, start=True, stop=True)
                aT = small.tile([PR, TB], F32, tag="aT")
                nc.scalar.activation(
                    aT, zT_ps, mybir.ActivationFunctionType.Sigmoid, bias=wb_sb
                )
                nc.vector.tensor_scalar(
                    aT,
                    aT,
                    float(A_MAX),
                    float(A_MIN),
                    op0=mybir.AluOpType.min,
                    op1=mybir.AluOpType.max,
                )

                # scan: s = a * s_prev + inj along T (chained across blocks)
                init = 0.0 if b == 0 else sT[:, t0 - 1 : t0]
                nc.vector.tensor_tensor_scan(
                    sT[:, t0 : t0 + TB],
                    aT,
                    injT_sb,
                    init,
                    op0=mybir.AluOpType.mult,
                    op1=mybir.AluOpType.add,
                )

                # t_tilde block: (s^T)^T @ V_o, one [128, P] tile per T-chunk
                for j in range(TB // (TC * PAIR)):
                    ps_o = ps_out.tile([128, P], F32, tag="out_ps")
                    ps_o2 = ps_out.tile([128, P], F32, tag="out_ps")
                    ob = outp.tile([128, PAIR, P], F32, tag="ob")
                    for a, pso in ((0, ps_o), (1, ps_o2)):
                        tc0 = t0 + (j * PAIR + a) * TC
                        for h in range(2):
                            nc.tensor.matmul(
                                pso[:, h * 512 : (h + 1) * 512],
                                lhsT=sT[:, tc0 : tc0 + TC],
                                rhs=vo_sb[:, h * 512 : (h + 1) * 512],
                                start=True,
                                stop=True,
                            )
                        if (b * 2 + j) % 2 == 0:
                            nc.vector.tensor_copy(ob[:, a, :], pso)
                        else:
                            nc.scalar.copy(ob[:, a, :], pso)
                    row0 = t0 + j * TC * PAIR
                    nc.sync.dma_start(
                        out=out_d[row0 : row0 + TC * PAIR, :].rearrange(
                            "(a k) d -> k a d", k=128
                        ),
                        in_=ob,
                    )

            nc.sync.dma_start(out=slast_d[:], in_=sT[:, T - 1 : T])

    if not nc.is_finalized():
        nc.finalize()
    return nc


_NC_CACHE = None


def _get_nc():
    global _NC_CACHE
    if _NC_CACHE is None:
        _NC_CACHE = _build()
    return _NC_CACHE


def kernel(t, F, V_r, V_b, V_o, W_w, W_b, _trace=False, _tmpdir=None):
    t = np.ascontiguousarray(np.asarray(t, dtype=np.float32))
    F = np.ascontiguousarray(np.asarray(F, dtype=np.float32))
    V_r = np.ascontiguousarray(np.asarray(V_r, dtype=np.float32))
    V_b = np.ascontiguousarray(np.asarray(V_b, dtype=np.float32))
    V_o = np.ascontiguousarray(np.asarray(V_o, dtype=np.float32))
    W_wT = np.ascontiguousarray(np.asarray(W_w, dtype=np.float32).T)
    W_b = np.ascontiguousarray(np.asarray(W_b, dtype=np.float32).reshape(PR, 1))

    nc = _get_nc()
    in_maps = [
        {
            "t": t[b],
            "F": F[b],
            "V_r": V_r,
            "V_b": V_b,
            "V_o": V_o,
            "W_wT": W_wT,
            "W_b": W_b,
        }
        for b in range(B)
    ]
    res = run_bass_kernel_spmd(
        nc, in_maps, core_ids=list(range(B)), trace=_trace, tmpdir=_tmpdir
    )
    LAST_RUN["exec_time_ns"] = res.exec_time_ns
    LAST_RUN["results"] = res

    t_tilde = np.stack([res.results[b]["t_tilde"] for b in range(B)])
    s_last = np.stack([res.results[b]["s_last"].reshape(PR) for b in range(B)])
    return t_tilde, s_last


# revision 17
# speedup vs baseline: 44294.9406x; 1.4204x over previous
"""Trainium2 Bass kernel for CeptaSSMLiteLowRank.

Computes, per batch b (B=8, T=2048, P=1024, P_r=64):
    r   = t @ V_r                                  (B,T,64)
    a   = clip(sigmoid(r @ W_w.T + W_b), .01, .995)
    inj = (F*t) @ V_b
    s_t = a_t * s_{t-1} + inj_t   (scan over T)
    out = s @ V_o, plus s_last

Sharding: data-parallel over B across 8 NeuronCores (1 batch/core), params
replicated. The T-scan runs on the DVE hardware scan op (tensor_tensor_scan)
in [P_r-partition, T-free] layout, fp32. The P-contraction matmuls need t
(and F*t) transposed; t/F are cast to bf16 during the DMA load (fp32 matmul
is 4x slower: LOW_HIGH 2-pass at 2 cyc/col), transposed on the PE in bf16,
and all projections run bf16 with fp32 PSUM accumulation. The a/scan path
(sigmoid, clip, scan state) stays fp32.
"""

import sys

if "/opt/trn_rl_repo" not in sys.path:
    sys.path.insert(0, "/opt/trn_rl_repo")

import numpy as np

import concourse.bacc as bacc
import concourse.bass as bass
import concourse.tile as tile
from concourse import mybir
from concourse.bass_utils import run_bass_kernel_spmd
from concourse.masks import make_identity

B, T, P, PR = 8, 2048, 1024, 64
A_MIN, A_MAX = 0.01, 0.995

F32 = mybir.dt.float32
BF16 = mybir.dt.bfloat16
TC = 128           # timestep tile (partition dim of natural tiles)
TB = 512           # timestep block (scan/matmul granularity)
NPC = P // 128     # 8 P-chunks
NB = T // TB       # 4 blocks
PAIR = 2           # T-tiles per DMA (1 MiB transfers)
N_WARM = 0         # HAM warm-up transposes

LAST_RUN = {}


def _build() -> bass.Bass:
    nc = bacc.Bacc()

    t_d = nc.declare_dram_parameter("t", [T, P], F32, isOutput=False)
    f_d = nc.declare_dram_parameter("F", [T, P], F32, isOutput=False)
    vr_d = nc.declare_dram_parameter("V_r", [P, PR], BF16, isOutput=False)
    vb_d = nc.declare_dram_parameter("V_b", [P, PR], BF16, isOutput=False)
    vo_d = nc.declare_dram_parameter("V_o", [PR, P], BF16, isOutput=False)
    wwt_d = nc.declare_dram_parameter("W_wT", [PR, PR], BF16, isOutput=False)
    wb_d = nc.declare_dram_parameter("W_b", [PR, 1], F32, isOutput=False)
    out_d = nc.declare_dram_parameter("t_tilde", [T, P], F32, isOutput=True)
    slast_d = nc.declare_dram_parameter("s_last", [PR, 1], F32, isOutput=True)

    with tile.TileContext(nc) as tc:
        with (
            tc.tile_pool(name="consts", bufs=1) as consts,
            tc.tile_pool(name="inp", bufs=3) as inp,
            tc.tile_pool(name="trans", bufs=2) as trans,
            tc.tile_pool(name="small", bufs=2) as small,
            tc.tile_pool(name="outp", bufs=3) as outp,
            tc.tile_pool(name="ps_tr", bufs=3, space="PSUM") as ps_tr,
            tc.tile_pool(name="ps_mm", bufs=2, space="PSUM") as ps_mm,
            tc.tile_pool(name="ps_out", bufs=3, space="PSUM") as ps_out,
        ):
            ident = consts.tile([128, 128], BF16)
            make_identity(nc, ident)

            # V_r/V_b as lhsT chunks: [K=128 (P-chunk), 8 chunks, M=64]
            vr_sb = consts.tile([128, NPC, PR], BF16)
            nc.sync.dma_start(
                out=vr_sb, in_=vr_d[:].rearrange("(c k) m -> k c m", k=128)
            )
            vb_sb = consts.tile([128, NPC, PR], BF16)
            nc.sync.dma_start(
                out=vb_sb, in_=vb_d[:].rearrange("(c k) m -> k c m", k=128)
            )
            vo_sb = consts.tile([PR, P], BF16)
            nc.sync.dma_start(out=vo_sb, in_=vo_d[:])
            wwt_sb = consts.tile([PR, PR], BF16)
            nc.sync.dma_start(out=wwt_sb, in_=wwt_d[:])
            wb_sb = consts.tile([PR, 1], F32)
            nc.sync.dma_start(out=wb_sb, in_=wb_d[:])

            if N_WARM:
                # HAM warm-up: keep the PE busy from t=0 so the clock gate is
                # at 8/8 by the time the first real tiles land (~3.4us window).
                warm_ps = ps_out.tile([128, 128], F32, tag="out_ps")
                for _ in range(N_WARM):
                    nc.tensor.matmul(
                        warm_ps, lhsT=ident, rhs=ident, start=True, stop=True
                    )

            # s^T accumulator for the whole sequence: [64, T] fp32
            sT = small.tile([PR, T], F32, bufs=1)

            cp_i = 0  # round-robin PSUM->SBUF copies between DVE and ACT

            for b in range(NB):
                t0 = b * TB

                # transposed bf16 tiles for this block: [128 (P-chunk), 8, TB]
                tT = trans.tile([128, NPC, TB], BF16, tag="tT")
                gT = trans.tile([128, NPC, TB], BF16, tag="gT")

                for j in range(TB // (TC * PAIR)):  # 2 pair-tiles per block
                    row0 = t0 + j * TC * PAIR
                    t_nat = inp.tile([128, PAIR, P], BF16, tag="t_nat")
                    nc.gpsimd.dma_start(  # SWDGE: fp32 -> bf16 cast in DMA
                        out=t_nat,
                        in_=t_d[row0 : row0 + TC * PAIR, :].rearrange(
                            "(a k) d -> k a d", k=128
                        ),
                    )
                    g_nat = inp.tile([128, PAIR, P], BF16, tag="g_nat")
                    nc.gpsimd.dma_start(
                        out=g_nat,
                        in_=f_d[row0 : row0 + TC * PAIR, :].rearrange(
                            "(a k) d -> k a d", k=128
                        ),
                    )
                    # G = F * t (elementwise) on gpsimd, in place over F
                    nc.gpsimd.tensor_mul(g_nat, g_nat, t_nat)

                    for a in range(PAIR):
                        ti = j * PAIR + a  # T-tile index within block (0..3)
                        for src, dstT, nm in ((t_nat, tT, "t"), (g_nat, gT, "g")):
                            # transpose = bf16 matmul against identity -> f32
                            # PSUM (single-pass, standard matmul path)
                            for grp in range(2):
                                ps = ps_tr.tile(
                                    [128, 512], F32, tag="ps_tr", name=f"ps_{nm}"
                                )
                                for c4 in range(4):
                                    c = grp * 4 + c4
                                    nc.tensor.matmul(
                                        ps[:, c4 * 128 : (c4 + 1) * 128],
                                        lhsT=src[:, a, c * 128 : (c + 1) * 128],
                                        rhs=ident,
                                        start=True,
                                        stop=True,
                                    )
                                # one engine per destination tensor, so each
                                # consumer matmul waits on a single semaphore
                                dst = dstT[
                                    :,
                                    grp * 4 : (grp + 1) * 4,
                                    ti * 128 : (ti + 1) * 128,
                                ]
                                psv = ps[:].rearrange("k (c m) -> k c m", c=4)
                                if nm == "t":
                                    nc.vector.tensor_copy(dst, psv)
                                else:
                                    nc.scalar.copy(dst, psv)

                # r^T and inj^T for this block: accumulate over 8 P-chunks
                rT_ps = ps_mm.tile([PR, TB], F32, tag="mm", name="rT_ps")
                for c in range(NPC):
                    nc.tensor.matmul(
                        rT_ps,
                        lhsT=vr_sb[:, c, :],
                        rhs=tT[:, c, :],
                        start=(c == 0),
                        stop=(c == NPC - 1),
                    )
                rT_sb = small.tile([PR, TB], BF16, tag="rT")
                nc.vector.tensor_copy(rT_sb, rT_ps)

                injT_ps = ps_mm.tile([PR, TB], F32, tag="mm", name="injT_ps")
                for c in range(NPC):
                    nc.tensor.matmul(
                        injT_ps,
                        lhsT=vb_sb[:, c, :],
                        rhs=gT[:, c, :],
                        start=(c == 0),
                        stop=(c == NPC - 1),
                    )
                injT_sb = small.tile([PR, TB], F32, tag="injT")
                nc.scalar.copy(injT_sb, injT_ps)

                # a^T = clip(sigmoid(W_w @ r^T + W_b)), fp32 values
                zT_ps = ps_mm.tile([PR, TB], F32, tag="mm", name="zT_ps")
                nc.tensor.matmul(zT_ps, lhsT=wwt_sb, rhs=rT_sb, start=True, stop=True)
                aT = small.tile([PR, TB], F32, tag="aT")
                nc.scalar.activation(
                    aT, zT_ps, mybir.ActivationFunctionType.Sigmoid, bias=wb_sb
                )
                nc.vector.tensor_scalar(
                    aT,
                    aT,
                    float(A_MAX),
                    float(A_MIN),
                    op0=mybir.AluOpType.min,
                    op1=mybir.AluOpType.max,
                )

                # scan: s = a * s_prev + inj along T (fp32, chained across blocks)
                init = 0.0 if b == 0 else sT[:, t0 - 1 : t0]
                nc.vector.tensor_tensor_scan(
                    sT[:, t0 : t0 + TB],
                    aT,
                    injT_sb,
                    init,
                    op0=mybir.AluOpType.mult,
                    op1=mybir.AluOpType.add,
                )
                # bf16 copy of s^T for the up-projection matmul
                sTb = small.tile([PR, TB], BF16, tag="sTb")
                nc.vector.tensor_copy(sTb, sT[:, t0 : t0 + TB])

                # t_tilde block: (s^T)^T @ V_o, one [128, 512] psum per half-tile
                for j in range(TB // TC):
                    tc0 = t0 + j * TC
                    ob = outp.tile([128, P], F32, tag="ob")
                    for h in range(2):
                        pso = ps_out.tile([128, 512], F32, tag="out_ps")
                        nc.tensor.matmul(
                            pso,
                            lhsT=sTb[:, j * TC : (j + 1) * TC],
                            rhs=vo_sb[:, h * 512 : (h + 1) * 512],
                            start=True,
                            stop=True,
                        )
                        if cp_i % 2 == 0:
                            nc.vector.tensor_copy(ob[:, h * 512 : (h + 1) * 512], pso)
                        else:
                            nc.scalar.copy(ob[:, h * 512 : (h + 1) * 512], pso)
                        cp_i += 1
                    nc.sync.dma_start(out=out_d[tc0 : tc0 + TC, :], in_=ob)

            nc.sync.dma_start(out=slast_d[:], in_=sT[:, T - 1 : T])

    if not nc.is_finalized():
        nc.finalize()
    return nc


_NC_CACHE = None


def _get_nc():
    global _NC_CACHE
    if _NC_CACHE is None:
        _NC_CACHE = _build()
    return _NC_CACHE


def kernel(t, F, V_r, V_b, V_o, W_w, W_b, _trace=False, _tmpdir=None):
    bf16 = mybir.dt.np(BF16)
    t = np.ascontiguousarray(np.asarray(t, dtype=np.float32))
    F = np.ascontiguousarray(np.asarray(F, dtype=np.float32))
    V_r = np.ascontiguousarray(np.asarray(V_r, dtype=np.float32).astype(bf16))
    V_b = np.ascontiguousarray(np.asarray(V_b, dtype=np.float32).astype(bf16))
    V_o = np.ascontiguousarray(np.asarray(V_o, dtype=np.float32).astype(bf16))
    W_wT = np.ascontiguousarray(np.asarray(W_w, dtype=np.float32).T.astype(bf16))
    W_b = np.ascontiguousarray(np.asarray(W_b, dtype=np.float32).reshape(PR, 1))

    nc = _get_nc()
    in_maps = [
        {
            "t": t[b],
            "F": F[b],
            "V_r": V_r,
            "V_b": V_b,
            "V_o": V_o,
            "W_wT": W_wT,
            "W_b": W_b,
        }
        for b in range(B)
    ]
    res = run_bass_kernel_spmd(
        nc, in_maps, core_ids=list(range(B)), trace=_trace, tmpdir=_tmpdir
    )
    LAST_RUN["exec_time_ns"] = res.exec_time_ns
    LAST_RUN["results"] = res

    t_tilde = np.stack([res.results[b]["t_tilde"] for b in range(B)])
    s_last = np.stack([res.results[b]["s_last"].reshape(PR) for b in range(B)])
    return t_tilde, s_last


# revision 19
# speedup vs baseline: 49522.0135x; 1.1180x over previous
"""Trainium2 Bass kernel for CeptaSSMLiteLowRank.

Computes, per batch b (B=8, T=2048, P=1024, P_r=64):
    r   = t @ V_r                                  (B,T,64)
    a   = clip(sigmoid(r @ W_w.T + W_b), .01, .995)
    inj = (F*t) @ V_b
    s_t = a_t * s_{t-1} + inj_t   (scan over T)
    out = s @ V_o, plus s_last

Sharding: data-parallel over B across 8 NeuronCores (1 batch/core), params
replicated. The T-scan runs on the DVE hardware scan op (tensor_tensor_scan)
in [P_r-partition, T-free] layout, fp32. The P-contraction matmuls need t
(and F*t) transposed; t/F are cast to bf16 during the DMA load (fp32 matmul
is 4x slower: LOW_HIGH 2-pass at 2 cyc/col), transposed on the PE in bf16,
and all projections run bf16 with fp32 PSUM accumulation. The a/scan path
(sigmoid, clip, scan state) stays fp32.
"""

import sys

if "/opt/trn_rl_repo" not in sys.path:
    sys.path.insert(0, "/opt/trn_rl_repo")

import numpy as np

import concourse.bacc as bacc
import concourse.bass as bass
import concourse.tile as tile
from concourse import mybir
from concourse.bass_utils import run_bass_kernel_spmd
from concourse.masks import make_identity

B, T, P, PR = 8, 2048, 1024, 64
A_MIN, A_MAX = 0.01, 0.995

F32 = mybir.dt.float32
BF16 = mybir.dt.bfloat16
TC = 128           # timestep tile (partition dim of natural tiles)
TB = 512           # timestep block (scan/matmul granularity)
NPC = P // 128     # 8 P-chunks
NB = T // TB       # 4 blocks
PAIR = 2           # T-tiles per DMA (1 MiB transfers)
N_WARM = 0         # HAM warm-up transposes

LAST_RUN = {}


def _build() -> bass.Bass:
    nc = bacc.Bacc()

    t_d = nc.declare_dram_parameter("t", [T, P], F32, isOutput=False)
    f_d = nc.declare_dram_parameter("F", [T, P], F32, isOutput=False)
    vr_d = nc.declare_dram_parameter("V_r", [P, PR], BF16, isOutput=False)
    vb_d = nc.declare_dram_parameter("V_b", [P, PR], BF16, isOutput=False)
    vo_d = nc.declare_dram_parameter("V_o", [PR, P], BF16, isOutput=False)
    wwt_d = nc.declare_dram_parameter("W_wT", [PR, PR], BF16, isOutput=False)
    wb_d = nc.declare_dram_parameter("W_b", [PR, 1], F32, isOutput=False)
    out_d = nc.declare_dram_parameter("t_tilde", [T, P], BF16, isOutput=True)
    slast_d = nc.declare_dram_parameter("s_last", [PR, 1], F32, isOutput=True)

    with tile.TileContext(nc) as tc:
        with (
            tc.tile_pool(name="consts", bufs=1) as consts,
            tc.tile_pool(name="inp", bufs=3) as inp,
            tc.tile_pool(name="trans", bufs=2) as trans,
            tc.tile_pool(name="small", bufs=2) as small,
            tc.tile_pool(name="outp", bufs=3) as outp,
            tc.tile_pool(name="ps_tr", bufs=3, space="PSUM") as ps_tr,
            tc.tile_pool(name="ps_mm", bufs=2, space="PSUM") as ps_mm,
            tc.tile_pool(name="ps_out", bufs=3, space="PSUM") as ps_out,
        ):
            ident = consts.tile([128, 128], BF16)
            make_identity(nc, ident)

            # V_r/V_b as lhsT chunks: [K=128 (P-chunk), 8 chunks, M=64]
            vr_sb = consts.tile([128, NPC, PR], BF16)
            nc.sync.dma_start(
                out=vr_sb, in_=vr_d[:].rearrange("(c k) m -> k c m", k=128)
            )
            vb_sb = consts.tile([128, NPC, PR], BF16)
            nc.sync.dma_start(
                out=vb_sb, in_=vb_d[:].rearrange("(c k) m -> k c m", k=128)
            )
            vo_sb = consts.tile([PR, P], BF16)
            nc.sync.dma_start(out=vo_sb, in_=vo_d[:])
            wwt_sb = consts.tile([PR, PR], BF16)
            nc.sync.dma_start(out=wwt_sb, in_=wwt_d[:])
            wb_sb = consts.tile([PR, 1], F32)
            nc.sync.dma_start(out=wb_sb, in_=wb_d[:])

            if N_WARM:
                # HAM warm-up: keep the PE busy from t=0 so the clock gate is
                # at 8/8 by the time the first real tiles land (~3.4us window).
                warm_ps = ps_out.tile([128, 128], F32, tag="out_ps")
                for _ in range(N_WARM):
                    nc.tensor.matmul(
                        warm_ps, lhsT=ident, rhs=ident, start=True, stop=True
                    )

            # s^T accumulator for the whole sequence: [64, T] fp32
            sT = small.tile([PR, T], F32, bufs=1)

            cp_i = 0  # round-robin PSUM->SBUF copies between DVE and ACT

            for b in range(NB):
                t0 = b * TB

                # transposed bf16 tiles for this block: [128 (P-chunk), 8, TB]
                tT = trans.tile([128, NPC, TB], BF16, tag="tT")
                gT = trans.tile([128, NPC, TB], BF16, tag="gT")

                for j in range(TB // (TC * PAIR)):  # 2 pair-tiles per block
                    row0 = t0 + j * TC * PAIR
                    t_nat = inp.tile([128, PAIR, P], BF16, tag="t_nat")
                    nc.gpsimd.dma_start(  # SWDGE: fp32 -> bf16 cast in DMA
                        out=t_nat,
                        in_=t_d[row0 : row0 + TC * PAIR, :].rearrange(
                            "(a k) d -> k a d", k=128
                        ),
                    )
                    g_nat = inp.tile([128, PAIR, P], BF16, tag="g_nat")
                    nc.gpsimd.dma_start(
                        out=g_nat,
                        in_=f_d[row0 : row0 + TC * PAIR, :].rearrange(
                            "(a k) d -> k a d", k=128
                        ),
                    )
                    # G = F * t (elementwise) on DVE, in place over F
                    nc.vector.tensor_mul(g_nat, g_nat, t_nat)

                    for a in range(PAIR):
                        ti = j * PAIR + a  # T-tile index within block (0..3)
                        for src, dstT, nm in ((t_nat, tT, "t"), (g_nat, gT, "g")):
                            # transpose = bf16 matmul against identity -> f32
                            # PSUM (single-pass, standard matmul path)
                            for grp in range(2):
                                ps = ps_tr.tile(
                                    [128, 512], F32, tag="ps_tr", name=f"ps_{nm}"
                                )
                                for c4 in range(4):
                                    c = grp * 4 + c4
                                    nc.tensor.matmul(
                                        ps[:, c4 * 128 : (c4 + 1) * 128],
                                        lhsT=src[:, a, c * 128 : (c + 1) * 128],
                                        rhs=ident,
                                        start=True,
                                        stop=True,
                                    )
                                dst = dstT[
                                    :,
                                    grp * 4 : (grp + 1) * 4,
                                    ti * 128 : (ti + 1) * 128,
                                ]
                                psv = ps[:].rearrange("k (c m) -> k c m", c=4)
                                # DVE also runs mul/scan/clip; give ACT 2/3
                                if cp_i % 3 == 0:
                                    nc.vector.tensor_copy(dst, psv)
                                else:
                                    nc.scalar.copy(dst, psv)
                                cp_i += 1

                # r^T and inj^T for this block: accumulate over 8 P-chunks
                rT_ps = ps_mm.tile([PR, TB], F32, tag="mm", name="rT_ps")
                for c in range(NPC):
                    nc.tensor.matmul(
                        rT_ps,
                        lhsT=vr_sb[:, c, :],
                        rhs=tT[:, c, :],
                        start=(c == 0),
                        stop=(c == NPC - 1),
                    )
                rT_sb = small.tile([PR, TB], BF16, tag="rT")
                nc.scalar.copy(rT_sb, rT_ps)

                injT_ps = ps_mm.tile([PR, TB], F32, tag="mm", name="injT_ps")
                for c in range(NPC):
                    nc.tensor.matmul(
                        injT_ps,
                        lhsT=vb_sb[:, c, :],
                        rhs=gT[:, c, :],
                        start=(c == 0),
                        stop=(c == NPC - 1),
                    )
                injT_sb = small.tile([PR, TB], F32, tag="injT")
                nc.scalar.copy(injT_sb, injT_ps)

                # a^T = clip(sigmoid(W_w @ r^T + W_b)), fp32 values
                zT_ps = ps_mm.tile([PR, TB], F32, tag="mm", name="zT_ps")
                nc.tensor.matmul(zT_ps, lhsT=wwt_sb, rhs=rT_sb, start=True, stop=True)
                aT = small.tile([PR, TB], F32, tag="aT")
                nc.scalar.activation(
                    aT, zT_ps, mybir.ActivationFunctionType.Sigmoid, bias=wb_sb
                )
                nc.vector.tensor_scalar(
                    aT,
                    aT,
                    float(A_MAX),
                    float(A_MIN),
                    op0=mybir.AluOpType.min,
                    op1=mybir.AluOpType.max,
                )

                # scan: s = a * s_prev + inj along T (fp32, chained across blocks)
                init = 0.0 if b == 0 else sT[:, t0 - 1 : t0]
                nc.vector.tensor_tensor_scan(
                    sT[:, t0 : t0 + TB],
                    aT,
                    injT_sb,
                    init,
                    op0=mybir.AluOpType.mult,
                    op1=mybir.AluOpType.add,
                )
                # bf16 copy of s^T for the up-projection matmul
                sTb = small.tile([PR, TB], BF16, tag="sTb")
                nc.scalar.copy(sTb, sT[:, t0 : t0 + TB])

                # t_tilde block: (s^T)^T @ V_o, one [128, 512] psum per half-tile
                for j in range(TB // TC):
                    tc0 = t0 + j * TC
                    ob = outp.tile([128, P], BF16, tag="ob")
                    for h in range(2):
                        pso = ps_out.tile([128, 512], F32, tag="out_ps")
                        nc.tensor.matmul(
                            pso,
                            lhsT=sTb[:, j * TC : (j + 1) * TC],
                            rhs=vo_sb[:, h * 512 : (h + 1) * 512],
                            start=True,
                            stop=True,
                        )
                        if cp_i % 3 == 0:
                            nc.vector.tensor_copy(ob[:, h * 512 : (h + 1) * 512], pso)
                        else:
                            nc.scalar.copy(ob[:, h * 512 : (h + 1) * 512], pso)
                        cp_i += 1
                    nc.sync.dma_start(out=out_d[tc0 : tc0 + TC, :], in_=ob)

            nc.sync.dma_start(out=slast_d[:], in_=sT[:, T - 1 : T])

    if not nc.is_finalized():
        nc.finalize()
    return nc


_NC_CACHE = None


def _get_nc():
    global _NC_CACHE
    if _NC_CACHE is None:
        _NC_CACHE = _build()
    return _NC_CACHE


def kernel(t, F, V_r, V_b, V_o, W_w, W_b, _trace=False, _tmpdir=None):
    bf16 = mybir.dt.np(BF16)
    t = np.ascontiguousarray(np.asarray(t, dtype=np.float32))
    F = np.ascontiguousarray(np.asarray(F, dtype=np.float32))
    V_r = np.ascontiguousarray(np.asarray(V_r, dtype=np.float32).astype(bf16))
    V_b = np.ascontiguousarray(np.asarray(V_b, dtype=np.float32).astype(bf16))
    V_o = np.ascontiguousarray(np.asarray(V_o, dtype=np.float32).astype(bf16))
    W_wT = np.ascontiguousarray(np.asarray(W_w, dtype=np.float32).T.astype(bf16))
    W_b = np.ascontiguousarray(np.asarray(W_b, dtype=np.float32).reshape(PR, 1))

    nc = _get_nc()
    in_maps = [
        {
            "t": t[b],
            "F": F[b],
            "V_r": V_r,
            "V_b": V_b,
            "V_o": V_o,
            "W_wT": W_wT,
            "W_b": W_b,
        }
        for b in range(B)
    ]
    res = run_bass_kernel_spmd(
        nc, in_maps, core_ids=list(range(B)), trace=_trace, tmpdir=_tmpdir
    )
    LAST_RUN["exec_time_ns"] = res.exec_time_ns
    LAST_RUN["results"] = res

    t_tilde = np.stack(
        [res.results[b]["t_tilde"].astype(np.float32) for b in range(B)]
    )
    s_last = np.stack([res.results[b]["s_last"].reshape(PR) for b in range(B)])
    return t_tilde, s_last


# revision 20
# speedup vs baseline: 55711.6082x; 1.1250x over previous
"""Trainium2 Bass kernel for CeptaSSMLiteLowRank.

Computes, per batch b (B=8, T=2048, P=1024, P_r=64):
    r   = t @ V_r                                  (B,T,64)
    a   = clip(sigmoid(r @ W_w.T + W_b), .01, .995)
    inj = (F*t) @ V_b
    s_t = a_t * s_{t-1} + inj_t   (scan over T)
    out = s @ V_o, plus s_last

Sharding: data-parallel over B across 8 NeuronCores (1 batch/core), params
replicated. The T-scan runs on the DVE hardware scan op (tensor_tensor_scan)
in [P_r-partition, T-free] layout, fp32. The P-contraction matmuls need t
(and F*t) transposed; t/F are cast to bf16 during the DMA load (fp32 matmul
is 4x slower: LOW_HIGH 2-pass at 2 cyc/col), transposed on the PE in bf16,
and all projections run bf16 with fp32 PSUM accumulation. The a/scan path
(sigmoid, clip, scan state) stays fp32.
"""

import sys

if "/opt/trn_rl_repo" not in sys.path:
    sys.path.insert(0, "/opt/trn_rl_repo")

import numpy as np

import concourse.bacc as bacc
import concourse.bass as bass
import concourse.tile as tile
from concourse import mybir
from concourse.bass_utils import run_bass_kernel_spmd
from concourse.masks import make_identity

B, T, P, PR = 8, 2048, 1024, 64
A_MIN, A_MAX = 0.01, 0.995

F32 = mybir.dt.float32
BF16 = mybir.dt.bfloat16
TC = 128           # timestep tile (partition dim of natural tiles)
TB = 512           # timestep block (scan/matmul granularity)
NPC = P // 128     # 8 P-chunks
NB = T // TB       # 4 blocks
PAIR = 2           # T-tiles per DMA (1 MiB transfers)
N_WARM = 0         # HAM warm-up transposes

LAST_RUN = {}


def _build() -> bass.Bass:
    nc = bacc.Bacc()

    t_d = nc.declare_dram_parameter("t", [T, P], F32, isOutput=False)
    f_d = nc.declare_dram_parameter("F", [T, P], F32, isOutput=False)
    vr_d = nc.declare_dram_parameter("V_r", [P, PR], BF16, isOutput=False)
    vb_d = nc.declare_dram_parameter("V_b", [P, PR], BF16, isOutput=False)
    vo_d = nc.declare_dram_parameter("V_o", [PR, P], BF16, isOutput=False)
    wwt_d = nc.declare_dram_parameter("W_wT", [PR, PR], BF16, isOutput=False)
    wb_d = nc.declare_dram_parameter("W_b", [PR, 1], F32, isOutput=False)
    out_d = nc.declare_dram_parameter("t_tilde", [T, P], BF16, isOutput=True)
    slast_d = nc.declare_dram_parameter("s_last", [PR, 1], F32, isOutput=True)

    with tile.TileContext(nc) as tc:
        with (
            tc.tile_pool(name="consts", bufs=1) as consts,
            tc.tile_pool(name="inp", bufs=3) as inp,
            tc.tile_pool(name="trans", bufs=2) as trans,
            tc.tile_pool(name="small", bufs=2) as small,
            tc.tile_pool(name="outp", bufs=3) as outp,
            tc.tile_pool(name="ps_tr", bufs=3, space="PSUM") as ps_tr,
            tc.tile_pool(name="ps_mm", bufs=2, space="PSUM") as ps_mm,
            tc.tile_pool(name="ps_out", bufs=3, space="PSUM") as ps_out,
        ):
            ident = consts.tile([128, 128], BF16)
            make_identity(nc, ident)

            # V_r/V_b as lhsT chunks: [K=128 (P-chunk), 8 chunks, M=64]
            vr_sb = consts.tile([128, NPC, PR], BF16)
            nc.sync.dma_start(
                out=vr_sb, in_=vr_d[:].rearrange("(c k) m -> k c m", k=128)
            )
            vb_sb = consts.tile([128, NPC, PR], BF16)
            nc.sync.dma_start(
                out=vb_sb, in_=vb_d[:].rearrange("(c k) m -> k c m", k=128)
            )
            vo_sb = consts.tile([PR, P], BF16)
            nc.sync.dma_start(out=vo_sb, in_=vo_d[:])
            wwt_sb = consts.tile([PR, PR], BF16)
            nc.sync.dma_start(out=wwt_sb, in_=wwt_d[:])
            wb_sb = consts.tile([PR, 1], F32)
            nc.sync.dma_start(out=wb_sb, in_=wb_d[:])

            if N_WARM:
                # HAM warm-up: keep the PE busy from t=0 so the clock gate is
                # at 8/8 by the time the first real tiles land (~3.4us window).
                warm_ps = ps_out.tile([128, 128], F32, tag="out_ps")
                for _ in range(N_WARM):
                    nc.tensor.matmul(
                        warm_ps, lhsT=ident, rhs=ident, start=True, stop=True
                    )

            # s^T accumulator for the whole sequence: [64, T] fp32
            sT = small.tile([PR, T], F32, bufs=1)

            cp = [0]  # round-robin PSUM->SBUF copies between DVE and ACT

            def copy_rr(dst, src):
                # DVE also runs the gT muls/scan/clip; give ACT ~3/5
                if cp[0] % 5 < 2:
                    nc.vector.tensor_copy(dst, src)
                else:
                    nc.scalar.copy(dst, src)
                cp[0] += 1

            def load_phase(b):
                """Input DMAs + transposes for block b; returns (tT, gT)."""
                t0 = b * TB
                # transposed bf16 tiles for this block: [128 (P-chunk), 8, TB]
                tT = trans.tile([128, NPC, TB], BF16, tag="tT")
                gT = trans.tile([128, NPC, TB], BF16, tag="gT")

                for j in range(TB // (TC * PAIR)):  # 2 pair-tiles per block
                    row0 = t0 + j * TC * PAIR
                    t_nat = inp.tile([128, PAIR, P], BF16, tag="t_nat")
                    nc.gpsimd.dma_start(  # SWDGE: fp32 -> bf16 cast in DMA
                        out=t_nat,
                        in_=t_d[row0 : row0 + TC * PAIR, :].rearrange(
                            "(a k) d -> k a d", k=128
                        ),
                    )
                    f_nat = inp.tile([128, PAIR, P], BF16, tag="f_nat")
                    nc.gpsimd.dma_start(
                        out=f_nat,
                        in_=f_d[row0 : row0 + TC * PAIR, :].rearrange(
                            "(a k) d -> k a d", k=128
                        ),
                    )

                    for a in range(PAIR):
                        ti = j * PAIR + a  # T-tile index within block (0..3)
                        # transpose = bf16 matmul against identity -> f32 PSUM
                        # (single-pass, standard matmul path)
                        for grp in range(2):
                            sl = (
                                slice(None),
                                slice(grp * 4, (grp + 1) * 4),
                                slice(ti * 128, (ti + 1) * 128),
                            )
                            ps_t = ps_tr.tile([128, 512], F32, tag="ps_tr")
                            for c4 in range(4):
                                c = grp * 4 + c4
                                nc.tensor.matmul(
                                    ps_t[:, c4 * 128 : (c4 + 1) * 128],
                                    lhsT=t_nat[:, a, c * 128 : (c + 1) * 128],
                                    rhs=ident,
                                    start=True,
                                    stop=True,
                                )
                            copy_rr(tT[sl], ps_t[:].rearrange("k (c m) -> k c m", c=4))
                            ps_f = ps_tr.tile([128, 512], F32, tag="ps_tr")
                            for c4 in range(4):
                                c = grp * 4 + c4
                                nc.tensor.matmul(
                                    ps_f[:, c4 * 128 : (c4 + 1) * 128],
                                    lhsT=f_nat[:, a, c * 128 : (c + 1) * 128],
                                    rhs=ident,
                                    start=True,
                                    stop=True,
                                )
                            # (F*t)^T = F^T * t^T fused into the PSUM drain (DVE)
                            nc.vector.tensor_mul(
                                gT[sl],
                                ps_f[:].rearrange("k (c m) -> k c m", c=4),
                                tT[sl],
                            )
                return tT, gT

            def math_phase(b, tT, gT):
                """Projections, gate, scan and output for block b."""
                t0 = b * TB
                # r^T and inj^T for this block: accumulate over 8 P-chunks
                rT_ps = ps_mm.tile([PR, TB], F32, tag="mm", name="rT_ps")
                for c in range(NPC):
                    nc.tensor.matmul(
                        rT_ps,
                        lhsT=vr_sb[:, c, :],
                        rhs=tT[:, c, :],
                        start=(c == 0),
                        stop=(c == NPC - 1),
                    )
                rT_sb = small.tile([PR, TB], BF16, tag="rT")
                nc.scalar.copy(rT_sb, rT_ps)

                injT_ps = ps_mm.tile([PR, TB], F32, tag="mm", name="injT_ps")
                for c in range(NPC):
                    nc.tensor.matmul(
                        injT_ps,
                        lhsT=vb_sb[:, c, :],
                        rhs=gT[:, c, :],
                        start=(c == 0),
                        stop=(c == NPC - 1),
                    )
                injT_sb = small.tile([PR, TB], F32, tag="injT")
                nc.scalar.copy(injT_sb, injT_ps)

                # a^T = clip(sigmoid(W_w @ r^T + W_b)), fp32 values
                zT_ps = ps_mm.tile([PR, TB], F32, tag="mm", name="zT_ps")
                nc.tensor.matmul(zT_ps, lhsT=wwt_sb, rhs=rT_sb, start=True, stop=True)
                aT = small.tile([PR, TB], F32, tag="aT")
                nc.scalar.activation(
                    aT, zT_ps, mybir.ActivationFunctionType.Sigmoid, bias=wb_sb
                )
                nc.vector.tensor_scalar(
                    aT,
                    aT,
                    float(A_MAX),
                    float(A_MIN),
                    op0=mybir.AluOpType.min,
                    op1=mybir.AluOpType.max,
                )

                # scan: s = a * s_prev + inj along T (fp32, chained across blocks)
                init = 0.0 if b == 0 else sT[:, t0 - 1 : t0]
                nc.vector.tensor_tensor_scan(
                    sT[:, t0 : t0 + TB],
                    aT,
                    injT_sb,
                    init,
                    op0=mybir.AluOpType.mult,
                    op1=mybir.AluOpType.add,
                )
                # bf16 copy of s^T for the up-projection matmul
                sTb = small.tile([PR, TB], BF16, tag="sTb")
                nc.scalar.copy(sTb, sT[:, t0 : t0 + TB])

                # t_tilde block: (s^T)^T @ V_o, one [128, 512] psum per half-tile
                for j in range(TB // TC):
                    tc0 = t0 + j * TC
                    ob = outp.tile([128, P], BF16, tag="ob")
                    for h in range(2):
                        pso = ps_out.tile([128, 512], F32, tag="out_ps")
                        nc.tensor.matmul(
                            pso,
                            lhsT=sTb[:, j * TC : (j + 1) * TC],
                            rhs=vo_sb[:, h * 512 : (h + 1) * 512],
                            start=True,
                            stop=True,
                        )
                        copy_rr(ob[:, h * 512 : (h + 1) * 512], pso)
                    nc.sync.dma_start(out=out_d[tc0 : tc0 + TC, :], in_=ob)

            # software pipeline: emit block b's loads/transposes before block
            # b-1's math so the in-order PE queue never stalls on the copies
            # (keeps the PE continuously busy -> HAM stays at 8/8)
            pending = None
            for b in range(NB):
                tg = load_phase(b)
                if pending is not None:
                    math_phase(b - 1, *pending)
                pending = tg
            math_phase(NB - 1, *pending)

            nc.sync.dma_start(out=slast_d[:], in_=sT[:, T - 1 : T])

    if not nc.is_finalized():
        nc.finalize()
    return nc


_NC_CACHE = None


def _get_nc():
    global _NC_CACHE
    if _NC_CACHE is None:
        _NC_CACHE = _build()
    return _NC_CACHE


def kernel(t, F, V_r, V_b, V_o, W_w, W_b, _trace=False, _tmpdir=None):
    bf16 = mybir.dt.np(BF16)
    t = np.ascontiguousarray(np.asarray(t, dtype=np.float32))
    F = np.ascontiguousarray(np.asarray(F, dtype=np.float32))
    V_r = np.ascontiguousarray(np.asarray(V_r, dtype=np.float32).astype(bf16))
    V_b = np.ascontiguousarray(np.asarray(V_b, dtype=np.float32).astype(bf16))
    V_o = np.ascontiguousarray(np.asarray(V_o, dtype=np.float32).astype(bf16))
    W_wT = np.ascontiguousarray(np.asarray(W_w, dtype=np.float32).T.astype(bf16))
    W_b = np.ascontiguousarray(np.asarray(W_b, dtype=np.float32).reshape(PR, 1))

    nc = _get_nc()
    in_maps = [
        {
            "t": t[b],
            "F": F[b],
            "V_r": V_r,
            "V_b": V_b,
            "V_o": V_o,
            "W_wT": W_wT,
            "W_b": W_b,
        }
        for b in range(B)
    ]
    res = run_bass_kernel_spmd(
        nc, in_maps, core_ids=list(range(B)), trace=_trace, tmpdir=_tmpdir
    )
    LAST_RUN["exec_time_ns"] = res.exec_time_ns
    LAST_RUN["results"] = res

    t_tilde = np.stack(
        [res.results[b]["t_tilde"].astype(np.float32) for b in range(B)]
    )
    s_last = np.stack([res.results[b]["s_last"].reshape(PR) for b in range(B)])
    return t_tilde, s_last
